# revision 7
# baseline (speedup 1.0000x reference)
"""CategoricalGCNEncoder on 8 Trainium2 NeuronCores (Bass/Tile).

Design ("v7"):
  - Nodes (dst) sharded across 8 cores; per-layer feature tables AllGathered.
  - All matmuls in bf16 (single PE pass). PSUM stays f32.
  - Embedding + first matmul fused: h1 = sum_f onehot_f @ T_f with
    T_f = emb_f @ W1_f computed on device (f32) then cast to bf16; one-hot
    encodings uploaded from host in bf16, window-contiguous layout.
  - GCN normalization folded into node scaling: ht = dis * h; out[d] =
    dis[d] * (sum_{e:dst=d} ht[src] + ht[d]) + b.
  - Edge phase: per-edge rows gathered with gpsimd.dma_gather in
    prepare_only mode + trigger_dma (desc-gen never blocks on transfers;
    all 4 SWDGE queues' transfers overlap). int16 idx, 256B bf16 rows.
  - Segment-sum: per dst-window (128 nodes) PSUM accumulation of
    matmul(lhsT=S_col[128x128] bf16, rhs=msg_col[128xF] bf16). S is built
    on-device from iota/dstrel via is_equal, split between the Vector and
    GpSimd engines to balance load. Pad edges carry dstrel=-1 -> zero
    column -> no contribution.
  - Node features live only in the bf16 staging/table rows; epilogue adds
    read them with mixed-dtype tensor_tensor (PSUM f32 + bf16).
  - Epilogue scalar math (dis scaling, relu, sqrt, (x-mu)*rstd) runs on
    the Scalar engine via activation scale/bias.
  - Host packs nodes into windows (vector bin packing) so every (window,
    bucket) has exactly 4 columns of 128 edge slots; the node->slot
    permutation is undone on the host at the end.
"""

import numpy as np
import ml_dtypes

import concourse.bass as bass
import concourse.mybir as mybir
import concourse.tile as tile
from concourse import bacc
from concourse.bass_utils import run_bass_kernel_spmd

BF16 = ml_dtypes.bfloat16

# ---------------- problem constants (hardcoded; kernel must be self-contained)
N = 100000
E = 1600000
NF = 8
EMB = 16
IN_DIM = 128
HID = 64
OUT = 32
NCAT = 100
EPS = 1e-5

NCORE = 8
SH = N // NCORE            # 12500 nodes per core
P = 128
W = 104                    # windows per core
SLOTS = W * P              # 13312 slots per core (>= SH)
KQ = 4                     # columns per (window, bucket)
NQ = 4                     # src buckets == SWDGE queues
COLS = W * KQ              # columns per bucket stream (416)
TOTCOL = NQ * COLS         # total columns (1664)
TOTPOS = TOTCOL * P        # total edge slots (212992)
TBL = NCORE * SLOTS        # table rows (106496)
BUCK = TBL // NQ           # bucket size (26624) < 32768
GW = 4                     # windows per gather group
CAP_Q = KQ * P             # 512 edge slots per (w, q)
ROWC = 128                 # table row width (bf16) -> 256B rows for gather

f32 = mybir.dt.float32
bf16 = mybir.dt.bfloat16
i16 = mybir.dt.int16

_CACHE = {}


# ------------------------------------------------------------------ program
def build_program():
    nc = bacc.Bacc(None, target_bir_lowering=False, debug=False,
                   num_devices=NCORE, num_swdge_queues=NQ,
                   dynamic_dma_scratch_size=16384)
    with tile.TileContext(nc) as tc:
        _build(nc, tc)
    nc.compile()
    return nc


def _build(nc, tc):
    AF = mybir.ActivationFunctionType
    ALU = mybir.AluOpType

    from contextlib import ExitStack
    ctx = ExitStack()
    dram = ctx.enter_context(tc.tile_pool(name="dram", bufs=1, space="DRAM"))
    const = ctx.enter_context(tc.tile_pool(name="const", bufs=1))
    oh_pool = ctx.enter_context(tc.tile_pool(name="ohp", bufs=3))
    msg_pool = ctx.enter_context(tc.tile_pool(name="msgp", bufs=12))
    s_pool = ctx.enter_context(tc.tile_pool(name="sp", bufs=4))
    epi_pool = ctx.enter_context(tc.tile_pool(name="epip", bufs=4))
    psum_mm = ctx.enter_context(tc.tile_pool(name="psmm", bufs=3, space="PSUM"))
    psum_tr = ctx.enter_context(tc.tile_pool(name="pstr", bufs=2, space="PSUM"))
    psum_w2 = ctx.enter_context(tc.tile_pool(name="psw2", bufs=2, space="PSUM"))

    def din(name, shape, dtype=f32):
        return dram.tile(shape, dtype, kind="ExternalInput", name=name,
                         uniquify=False)

    # ---- inputs
    onehot = din("onehot", [W, NCAT, NF, P], bf16)
    idxs = din("idxs", [P, TOTPOS // 16], i16)
    dstrel = din("dstrel", [P, TOTCOL], bf16)
    degin = din("deg", [P, W])
    embT = din("embT", [EMB, NF * NCAT])
    w1 = din("w1", [EMB, NF, HID])
    w2 = din("w2", [HID, OUT], bf16)
    b1r = din("b1r", [P, HID])
    g1r = din("g1r", [P, HID])
    be1r = din("be1r", [P, HID])
    b2r = din("b2r", [P, OUT])
    g2r = din("g2r", [P, OUT])
    be2r = din("be2r", [P, OUT])
    iotain = din("iota", [P, P], bf16)
    identin = din("ident", [P, P])

    outx = dram.tile([SLOTS, OUT], f32, kind="ExternalOutput", name="outx",
                     uniquify=False)

    bounce1 = dram.tile([SLOTS, ROWC], bf16)
    table1 = dram.tile([TBL, ROWC], bf16, addr_space="Shared")
    bounce2 = dram.tile([SLOTS, ROWC], bf16)
    table2 = dram.tile([TBL, ROWC], bf16, addr_space="Shared")

    # ---- static SBUF
    idx_sb = const.tile([P, TOTPOS // 16], i16)
    nc.sync.dma_start(out=idx_sb[:], in_=idxs[:])
    dstrel_sb = const.tile([P, TOTCOL], bf16)
    nc.sync.dma_start(out=dstrel_sb[:], in_=dstrel[:])
    iota_sb = const.tile([P, P], bf16)
    nc.sync.dma_start(out=iota_sb[:], in_=iotain[:])
    ident_sb = const.tile([P, P], f32)
    nc.sync.dma_start(out=ident_sb[:], in_=identin[:])
    w1_sb = const.tile([EMB, NF, HID], f32)
    nc.sync.dma_start(out=w1_sb[:], in_=w1[:])
    w2_sb = const.tile([HID, OUT], bf16)
    nc.sync.dma_start(out=w2_sb[:], in_=w2[:])
    embT_sb = const.tile([EMB, NF * NCAT], f32)
    nc.sync.dma_start(out=embT_sb[:], in_=embT[:])
    b1_sb = const.tile([P, HID], f32)
    nc.sync.dma_start(out=b1_sb[:], in_=b1r[:])
    g1_sb = const.tile([P, HID], f32)
    nc.sync.dma_start(out=g1_sb[:], in_=g1r[:])
    be1_sb = const.tile([P, HID], f32)
    nc.sync.dma_start(out=be1_sb[:], in_=be1r[:])
    b2_sb = const.tile([P, OUT], f32)
    nc.sync.dma_start(out=b2_sb[:], in_=b2r[:])
    g2_sb = const.tile([P, OUT], f32)
    nc.sync.dma_start(out=g2_sb[:], in_=g2r[:])
    be2_sb = const.tile([P, OUT], f32)
    nc.sync.dma_start(out=be2_sb[:], in_=be2r[:])
    eps_sb = const.tile([P, 1], f32)
    nc.vector.memset(eps_sb[:], EPS)

    # per-queue DMA completion semaphores for prepare_only gathers
    qsems = [nc.alloc_semaphore(f"swdge_dma{q}") for q in range(NQ)]

    # dis = 1/sqrt(deg)
    deg_sb = const.tile([P, W], f32)
    nc.sync.dma_start(out=deg_sb[:], in_=degin[:])
    dis_sb = const.tile([P, W], f32)
    nc.scalar.activation(out=dis_sb[:], in_=deg_sb[:], func=AF.Sqrt)
    nc.vector.reciprocal(out=dis_sb[:], in_=dis_sb[:])

    # ---- T_f = emb_f @ W1_f  -> T_sb [NCAT, NF, HID] bf16
    T_sb = const.tile([NCAT, NF, HID], bf16)
    for f in range(NF):
        pt = psum_mm.tile([NCAT, HID], f32, space="PSUM", tag="ps")
        nc.tensor.matmul(
            out=pt[:],
            lhsT=embT_sb[:, f * NCAT:(f + 1) * NCAT],
            rhs=w1_sb[:, f, :],
            start=True, stop=True,
        )
        nc.scalar.copy(out=T_sb[:, f, :], in_=pt[:])

    # staging for the AllGather table rows (bf16, 256B rows)
    staging = const.tile([P, W, ROWC], bf16)
    nc.vector.memset(staging[:], 0.0)

    # ---- embedding: staging[p, w, :64] = dis * sum_f onehot_f_w.T @ T_f
    for w in range(W):
        oh = oh_pool.tile([NCAT, NF, P], bf16, tag="oh")
        nc.sync.dma_start(out=oh[:], in_=onehot[w])
        pe = psum_mm.tile([P, HID], f32, space="PSUM", tag="ps")
        for f in range(NF):
            nc.tensor.matmul(
                out=pe[:], lhsT=oh[:, f, :], rhs=T_sb[:, f, :],
                start=(f == 0), stop=(f == NF - 1),
            )
        nc.scalar.activation(out=staging[:, w, :HID], in_=pe[:], func=AF.Copy,
                             scale=dis_sb[:, w:w + 1])

    # bounce + allgather layer-1 table
    nc.sync.dma_start(
        out=bounce1.rearrange("(w p) h -> p w h", p=P), in_=staging[:])
    nc.gpsimd.collective_compute(
        "AllGather", mybir.AluOpType.bypass,
        replica_groups=[list(range(NCORE))],
        ins=[bounce1[:]], outs=[table1[:]],
    )

    def build_s(w, engine):
        """S[p, (q,c), j] = (dstrel[p, col(q,w,c)] == j), bf16."""
        s = s_pool.tile([P, NQ * KQ, P], bf16, tag="s")
        engine.tensor_tensor(
            out=s.rearrange("p (q c) j -> p q c j", q=NQ),
            in0=iota_sb.rearrange("p (o1 o2 j) -> p o1 o2 j", o1=1, o2=1)
                .to_broadcast([P, NQ, KQ, P]),
            in1=dstrel_sb.rearrange("p (q w c) -> p q w c", q=NQ, w=W)
                [:, :, w, :]
                .rearrange("p q (c o) -> p q c o", o=1)
                .to_broadcast([P, NQ, KQ, P]),
            op=ALU.is_equal,
        )
        return s

    gcnt = [0] * NQ   # gathers triggered per queue (cumulative, both layers)

    def edge_layer(table, fdim, epilogue):
        """Gather+segment-sum over all edges; call epilogue(w, psum_tile)."""
        ngrp = W // GW
        for g in range(ngrp):
            msgs = []
            for q in range(NQ):
                m = msg_pool.tile([P, GW * KQ, ROWC], bf16, tag="msg")
                c0 = (q * W + g * GW) * KQ          # first column of chunk
                nc.gpsimd.dma_gather(
                    m[:], table[BUCK * q:BUCK * (q + 1), :],
                    idx_sb[:, c0 * 8:(c0 + GW * KQ) * 8],
                    num_idxs=GW * KQ * P, num_idxs_reg=GW * KQ * P,
                    elem_size=ROWC, single_packet=False, queue_num=q,
                    prepare_only=True, sem=qsems[q],
                )
                msgs.append(m)
            for q in range(NQ):
                nc.gpsimd.trigger_dma(count=None, queue_num=q)
                gcnt[q] += 1
            for q in range(NQ):
                # SDMA bumps qsems[q] by 16 when a gather's transfer lands;
                # Tile only syncs on desc-gen, so gate the matmuls manually.
                nc.tensor.wait_ge(qsems[q], 16 * gcnt[q])
            for wi in range(GW):
                w = g * GW + wi
                s = build_s(w, nc.vector)
                pt = psum_mm.tile([P, fdim], f32, space="PSUM", tag="ps")
                k = 0
                for q in range(NQ):
                    for c in range(KQ):
                        nc.tensor.matmul(
                            out=pt[:],
                            lhsT=s[:, q * KQ + c, :],
                            rhs=msgs[q][:, wi * KQ + c, :fdim],
                            start=(k == 0), stop=(k == NQ * KQ - 1),
                        )
                        k += 1
                epilogue(w, pt)

    def layer_norm_core(x, tag):
        """Compute LN stats of x [P, fdim] f32; return (rstd, negmurstd)."""
        stats = epi_pool.tile([P, 1, 6], f32, tag=tag + "st")
        mv = epi_pool.tile([P, 2], f32, tag=tag + "mv")
        nc.vector.bn_stats(out=stats[:, 0, :], in_=x[:])
        nc.vector.bn_aggr(out=mv[:], in_=stats[:])
        rstd = epi_pool.tile([P, 1], f32, tag=tag + "rs")
        nc.scalar.activation(out=rstd[:], in_=mv[:, 1:2], func=AF.Sqrt,
                             bias=eps_sb[:], scale=1.0)
        nc.vector.reciprocal(out=rstd[:], in_=rstd[:])
        mr = epi_pool.tile([P, 1], f32, tag=tag + "mr")
        nc.vector.tensor_tensor(out=mr[:], in0=mv[:, 0:1], in1=rstd[:],
                                op=ALU.mult)
        nmr = epi_pool.tile([P, 1], f32, tag=tag + "nm")
        nc.scalar.activation(out=nmr[:], in_=mr[:], func=AF.Copy, scale=-1.0)
        return rstd, nmr

    def epi1(w, pt):
        # out1 = dis*(psum + ht_self) + b1 ; relu
        z = epi_pool.tile([P, HID], f32, tag="z1")
        nc.vector.tensor_tensor(out=z[:], in0=pt[:], in1=staging[:, w, :HID],
                                op=ALU.add)
        t = epi_pool.tile([P, HID], f32, tag="t1")
        nc.scalar.activation(out=t[:], in_=z[:], func=AF.Copy,
                             scale=dis_sb[:, w:w + 1])
        nc.vector.tensor_add(out=t[:], in0=t[:], in1=b1_sb[:])
        x = epi_pool.tile([P, HID], f32, tag="x1")
        nc.scalar.activation(out=x[:], in_=t[:], func=AF.Relu)
        # layernorm
        rstd, nmr = layer_norm_core(x, "a")
        xn = epi_pool.tile([P, HID], f32, tag="xn1")
        nc.scalar.activation(out=xn[:], in_=x[:], func=AF.Identity,
                             scale=rstd[:], bias=nmr[:])
        y = epi_pool.tile([P, HID], f32, tag="y1")
        nc.vector.tensor_tensor(out=y[:], in0=xn[:], in1=g1_sb[:],
                                op=ALU.mult)
        nc.vector.tensor_add(out=y[:], in0=y[:], in1=be1_sb[:])
        # h2 = dis * (y @ W2): transpose y then matmul (bf16)
        ptr = psum_tr.tile([HID, P], f32, space="PSUM", tag="tr")
        nc.tensor.transpose(out=ptr[:], in_=y[:], identity=ident_sb[:])
        yT = epi_pool.tile([HID, P], bf16, tag="yT")
        nc.scalar.copy(out=yT[:], in_=ptr[:])
        pw2 = psum_w2.tile([P, OUT], f32, space="PSUM", tag="w2")
        nc.tensor.matmul(out=pw2[:], lhsT=yT[:], rhs=w2_sb[:],
                         start=True, stop=True)
        nc.scalar.activation(out=staging[:, w, :OUT], in_=pw2[:],
                             func=AF.Copy, scale=dis_sb[:, w:w + 1])

    final = const.tile([P, W, OUT], f32)

    def epi2(w, pt):
        z = epi_pool.tile([P, OUT], f32, tag="z2")
        nc.vector.tensor_tensor(out=z[:], in0=pt[:], in1=staging[:, w, :OUT],
                                op=ALU.add)
        x = epi_pool.tile([P, OUT], f32, tag="x2")
        nc.scalar.activation(out=x[:], in_=z[:], func=AF.Copy,
                             scale=dis_sb[:, w:w + 1])
        nc.vector.tensor_add(out=x[:], in0=x[:], in1=b2_sb[:])
        rstd, nmr = layer_norm_core(x, "b")
        xn = epi_pool.tile([P, OUT], f32, tag="xn2")
        nc.scalar.activation(out=xn[:], in_=x[:], func=AF.Identity,
                             scale=rstd[:], bias=nmr[:])
        y = epi_pool.tile([P, OUT], f32, tag="y2")
        nc.vector.tensor_tensor(out=y[:], in0=xn[:], in1=g2_sb[:],
                                op=ALU.mult)
        nc.vector.tensor_add(out=final[:, w, :], in0=y[:], in1=be2_sb[:])

    # ---- layer 1
    edge_layer(table1, HID, epi1)

    # reset cols 32:64 so layer-2 rows (h2 in cols 0:32) are clean
    nc.vector.memset(staging[:, :, OUT:HID], 0.0)

    # bounce + allgather layer-2 table
    nc.sync.dma_start(
        out=bounce2.rearrange("(w p) h -> p w h", p=P), in_=staging[:])
    nc.gpsimd.collective_compute(
        "AllGather", mybir.AluOpType.bypass,
        replica_groups=[list(range(NCORE))],
        ins=[bounce2[:]], outs=[table2[:]],
    )

    # ---- layer 2
    edge_layer(table2, OUT, epi2)

    nc.sync.dma_start(
        out=outx.rearrange("(w p) o -> p w o", p=P), in_=final[:])
    ctx.close()


# ------------------------------------------------------------------ host prep
def _pack_core(dloc, q_of_edge):
    """Assign local nodes to (window, slot) with per-(w,q) capacity CAP_Q and
    <=P nodes per window.  Returns win[SH], pslot[SH]."""
    cnt = np.zeros((SH, NQ), np.int64)
    np.add.at(cnt, (dloc, q_of_edge), 1)
    tot = cnt.sum(1)
    order = np.argsort(-tot, kind="stable")
    fills = np.zeros((W, NQ), np.int64)
    counts = np.zeros(W, np.int64)
    win = np.zeros(SH, np.int64)
    for n in order:
        c = cnt[n]
        ok = (counts < P) & np.all(fills + c <= CAP_Q, axis=1)
        if not ok.any():
            raise RuntimeError("window packing failed")
        load = np.where(ok[:, None], fills + c, 1 << 30).max(axis=1)
        wsel = int(np.argmin(load))
        win[n] = wsel
        fills[wsel] += c
        counts[wsel] += 1
    pslot = np.zeros(SH, np.int64)
    for wsel in range(W):
        nodes = np.nonzero(win == wsel)[0]
        pslot[nodes] = np.arange(len(nodes))
    return win, pslot


def _host_prep(x_cat, edge_index, emb_tables, W1, b1, W2, b2,
               gamma1, beta1, gamma2, beta2):
    src = np.asarray(edge_index[0], np.int64)
    dst = np.asarray(edge_index[1], np.int64)
    deg = np.bincount(dst, minlength=N).astype(np.float64) + 1.0

    core_of = np.arange(N) // SH
    wins = np.zeros(N, np.int64)
    pslots = np.zeros(N, np.int64)
    srcq = src // (2 * SH)  # bucket of an edge = pair-of-cores owning src
    for k in range(NCORE):
        m = (dst // SH) == k
        dloc = dst[m] - k * SH
        win, ps = _pack_core(dloc, srcq[m])
        wins[k * SH:(k + 1) * SH] = win
        pslots[k * SH:(k + 1) * SH] = ps
    slot_of = wins * P + pslots               # slot within owner core
    trow = core_of * SLOTS + slot_of          # global table row

    in_maps = []
    perm_slots = []
    for k in range(NCORE):
        m = (dst // SH) == k
        es, ed = src[m], dst[m] - k * SH
        ew = wins[ed + k * SH]
        ep = pslots[ed + k * SH]
        eq = trow[es] // BUCK
        # stream position: per (q, w) block of CAP_Q slots, fill in order
        gkey = eq * W + ew
        order = np.argsort(gkey, kind="stable")
        gsort = gkey[order]
        start = np.searchsorted(gsort, np.arange(NQ * W))
        rank = np.arange(len(gsort)) - start[gsort]
        assert (rank < CAP_Q).all()
        pos = gsort * CAP_Q + rank
        idx16 = np.zeros(TOTPOS, np.int16)
        drel = np.full(TOTPOS, -1.0, np.float32)
        idx16[pos] = (trow[es][order] - eq[order] * BUCK).astype(np.int16)
        drel[pos] = ep[order].astype(np.float32)
        # wrap idx: j -> [j%16, j//16], replicate x8 partition groups
        idxw = np.tile(idx16.reshape(-1, 16).T, (8, 1))
        drelw = np.ascontiguousarray(drel.reshape(-1, P).T).astype(BF16)

        # onehot [W, NCAT, NF, P] bf16 for this core's slots
        oh = np.zeros((W, NCAT, NF, P), BF16)
        sl = slot_of[k * SH:(k + 1) * SH]
        xc = np.asarray(x_cat[k * SH:(k + 1) * SH], np.int64)
        wv = sl // P
        pv = sl % P
        for f in range(NF):
            oh[wv, xc[:, f], f, pv] = 1.0

        degs = np.ones(SLOTS, np.float32)
        degs[sl] = deg[k * SH:(k + 1) * SH]
        degw = np.ascontiguousarray(degs.reshape(W, P).T)

        embT = np.ascontiguousarray(
            np.asarray(emb_tables, np.float32).transpose(2, 0, 1)
            .reshape(EMB, NF * NCAT))

        rep = lambda v, d: np.broadcast_to(
            np.asarray(v, np.float32).reshape(1, d), (P, d)).copy()

        in_maps.append({
            "onehot": oh,
            "idxs": idxw,
            "dstrel": drelw,
            "deg": degw,
            "embT": embT,
            "w1": np.ascontiguousarray(np.asarray(W1, np.float32).reshape(NF, EMB, HID).transpose(1, 0, 2)),
            "w2": np.asarray(W2, np.float32).astype(BF16),
            "b1r": rep(b1, HID), "g1r": rep(gamma1, HID),
            "be1r": rep(beta1, HID),
            "b2r": rep(b2, OUT), "g2r": rep(gamma2, OUT),
            "be2r": rep(beta2, OUT),
            "iota": np.broadcast_to(np.arange(P, dtype=np.float32), (P, P)).astype(BF16).copy(),
            "ident": np.eye(P, dtype=np.float32),
        })
        perm_slots.append(sl)
    return in_maps, perm_slots


# ------------------------------------------------------------------ entry
def kernel(x_cat, edge_index, emb_tables, W1, b1, W2, b2,
           gamma1, beta1, gamma2, beta2, _res_hook=None):
    if "nc" not in _CACHE:
        _CACHE["nc"] = build_program()
    nc = _CACHE["nc"]
    in_maps, perm_slots = _host_prep(
        np.asarray(x_cat), np.asarray(edge_index), np.asarray(emb_tables),
        np.asarray(W1), np.asarray(b1), np.asarray(W2), np.asarray(b2),
        np.asarray(gamma1), np.asarray(beta1), np.asarray(gamma2),
        np.asarray(beta2))
    res = run_bass_kernel_spmd(nc, in_maps, list(range(NCORE)),
                               **(_res_hook or {}))
    out = np.empty((N, OUT), np.float32)
    for k in range(NCORE):
        full = res.results[k]["outx"]        # [SLOTS, OUT] slot-ordered
        out[k * SH:(k + 1) * SH] = full[perm_slots[k]]
    if _res_hook is not None:
        _res_hook["result"] = res
    return out


# revision 16
# speedup vs baseline: 1.6538x; 1.6538x over previous
"""CategoricalGCNEncoder on 8 Trainium2 NeuronCores (Bass/Tile).

Design ("v7"):
  - Nodes (dst) sharded across 8 cores; per-layer feature tables AllGathered.
  - All matmuls in bf16 (single PE pass). PSUM stays f32.
  - Embedding + first matmul fused: h1 = sum_f onehot_f @ T_f with
    T_f = emb_f @ W1_f computed on device (f32) then cast to bf16; one-hot
    encodings uploaded from host in bf16, window-contiguous layout.
  - GCN normalization folded into node scaling: ht = dis * h; out[d] =
    dis[d] * (sum_{e:dst=d} ht[src] + ht[d]) + b.
  - Edge phase: per-edge rows gathered with gpsimd.dma_gather in
    prepare_only mode + trigger_dma (desc-gen never blocks on transfers;
    all 4 SWDGE queues' transfers overlap). int16 idx, 256B bf16 rows.
  - Segment-sum: per dst-window (128 nodes) PSUM accumulation of
    matmul(lhsT=S_col[128x128] bf16, rhs=msg_col[128xF] bf16). S is built
    on-device from iota/dstrel via is_equal, split between the Vector and
    GpSimd engines to balance load. Pad edges carry dstrel=-1 -> zero
    column -> no contribution.
  - Node features live only in the bf16 staging/table rows; epilogue adds
    read them with mixed-dtype tensor_tensor (PSUM f32 + bf16).
  - Epilogue scalar math (dis scaling, relu, sqrt, (x-mu)*rstd) runs on
    the Scalar engine via activation scale/bias.
  - Host packs nodes into windows (vector bin packing) so every (window,
    bucket) has exactly 4 columns of 128 edge slots; the node->slot
    permutation is undone on the host at the end.
"""

import numpy as np
import ml_dtypes

import concourse.bass as bass
import concourse.mybir as mybir
import concourse.tile as tile
from concourse import bacc
from concourse.bass_utils import run_bass_kernel_spmd

BF16 = ml_dtypes.bfloat16

# ---------------- problem constants (hardcoded; kernel must be self-contained)
N = 100000
E = 1600000
NF = 8
EMB = 16
IN_DIM = 128
HID = 64
OUT = 32
NCAT = 100
EPS = 1e-5

NCORE = 8
SH = N // NCORE            # 12500 nodes per core
P = 128
W = 104                    # windows per core
SLOTS = W * P              # 13312 slots per core (>= SH)
KQ = 4                     # columns per (window, bucket)
NQ = 4                     # src buckets == SWDGE queues
COLS = W * KQ              # columns per bucket stream (416)
TOTCOL = NQ * COLS         # total columns (1664)
TOTPOS = TOTCOL * P        # total edge slots (212992)
TBL = NCORE * SLOTS        # table rows (106496)
BUCK = TBL // NQ           # bucket size (26624) < 32768
GW = 4                     # windows per gather group
CAP_Q = KQ * P             # 512 edge slots per (w, q)
ROWC = 128                 # table row width (bf16) -> 256B rows for gather

f32 = mybir.dt.float32
bf16 = mybir.dt.bfloat16
i16 = mybir.dt.int16

_CACHE = {}


# ------------------------------------------------------------------ program
def build_program():
    nc = bacc.Bacc(None, target_bir_lowering=False, debug=False,
                   num_devices=NCORE, num_swdge_queues=NQ,
                   dynamic_dma_scratch_size=16384)
    with tile.TileContext(nc) as tc:
        _build(nc, tc)
    nc.compile()
    return nc


def _build(nc, tc):
    AF = mybir.ActivationFunctionType
    ALU = mybir.AluOpType

    from contextlib import ExitStack
    ctx = ExitStack()
    dram = ctx.enter_context(tc.tile_pool(name="dram", bufs=1, space="DRAM"))
    const = ctx.enter_context(tc.tile_pool(name="const", bufs=1))
    oh_pool = ctx.enter_context(tc.tile_pool(name="ohp", bufs=3))
    msg_pool = ctx.enter_context(tc.tile_pool(name="msgp", bufs=12))
    s_pool = ctx.enter_context(tc.tile_pool(name="sp", bufs=4))
    epi_pool = ctx.enter_context(tc.tile_pool(name="epip", bufs=4))
    psum_mm = ctx.enter_context(tc.tile_pool(name="psmm", bufs=3, space="PSUM"))
    psum_tr = ctx.enter_context(tc.tile_pool(name="pstr", bufs=2, space="PSUM"))
    psum_w2 = ctx.enter_context(tc.tile_pool(name="psw2", bufs=2, space="PSUM"))

    def din(name, shape, dtype=f32):
        return dram.tile(shape, dtype, kind="ExternalInput", name=name,
                         uniquify=False)

    # ---- inputs
    onehot = din("onehot", [W, NCAT, NF, P], bf16)
    idxs = din("idxs", [P, TOTPOS // 16], i16)
    dstrel = din("dstrel", [P, TOTCOL], bf16)
    degin = din("deg", [P, W])
    embT = din("embT", [EMB, NF * NCAT])
    w1 = din("w1", [EMB, NF, HID])
    w2 = din("w2", [HID, OUT], bf16)
    b1r = din("b1r", [P, HID])
    g1r = din("g1r", [P, HID])
    be1r = din("be1r", [P, HID])
    b2r = din("b2r", [P, OUT])
    g2r = din("g2r", [P, OUT])
    be2r = din("be2r", [P, OUT])
    iotain = din("iota", [P, P], bf16)
    identin = din("ident", [P, P])

    outx = dram.tile([SLOTS, OUT], f32, kind="ExternalOutput", name="outx",
                     uniquify=False)

    bounce1 = dram.tile([SLOTS, ROWC], bf16)
    table1 = dram.tile([TBL, ROWC], bf16, addr_space="Shared")
    bounce2 = dram.tile([SLOTS, ROWC], bf16)
    table2 = dram.tile([TBL, ROWC], bf16, addr_space="Shared")

    # ---- static SBUF
    idx_sb = const.tile([P, TOTPOS // 16], i16)
    nc.sync.dma_start(out=idx_sb[:], in_=idxs[:])
    dstrel_sb = const.tile([P, TOTCOL], bf16)
    nc.sync.dma_start(out=dstrel_sb[:], in_=dstrel[:])
    iota_sb = const.tile([P, P], bf16)
    nc.sync.dma_start(out=iota_sb[:], in_=iotain[:])
    ident_sb = const.tile([P, P], f32)
    nc.sync.dma_start(out=ident_sb[:], in_=identin[:])
    w1_sb = const.tile([EMB, NF, HID], f32)
    nc.sync.dma_start(out=w1_sb[:], in_=w1[:])
    w2_sb = const.tile([HID, OUT], bf16)
    nc.sync.dma_start(out=w2_sb[:], in_=w2[:])
    embT_sb = const.tile([EMB, NF * NCAT], f32)
    nc.sync.dma_start(out=embT_sb[:], in_=embT[:])
    b1_sb = const.tile([P, HID], f32)
    nc.sync.dma_start(out=b1_sb[:], in_=b1r[:])
    g1_sb = const.tile([P, HID], f32)
    nc.sync.dma_start(out=g1_sb[:], in_=g1r[:])
    be1_sb = const.tile([P, HID], f32)
    nc.sync.dma_start(out=be1_sb[:], in_=be1r[:])
    b2_sb = const.tile([P, OUT], f32)
    nc.sync.dma_start(out=b2_sb[:], in_=b2r[:])
    g2_sb = const.tile([P, OUT], f32)
    nc.sync.dma_start(out=g2_sb[:], in_=g2r[:])
    be2_sb = const.tile([P, OUT], f32)
    nc.sync.dma_start(out=be2_sb[:], in_=be2r[:])
    eps_sb = const.tile([P, 1], f32)
    nc.vector.memset(eps_sb[:], EPS)

    # per-queue DMA completion semaphores for prepare_only gathers
    qsems = [nc.alloc_semaphore(f"swdge_dma{q}") for q in range(NQ)]

    # dis = 1/sqrt(deg)
    deg_sb = const.tile([P, W], f32)
    nc.sync.dma_start(out=deg_sb[:], in_=degin[:])
    dis_sb = const.tile([P, W], f32)
    nc.scalar.activation(out=dis_sb[:], in_=deg_sb[:], func=AF.Sqrt)
    nc.vector.reciprocal(out=dis_sb[:], in_=dis_sb[:])

    # ---- T_f = emb_f @ W1_f  -> T_sb [NCAT, NF, HID] bf16
    T_sb = const.tile([NCAT, NF, HID], bf16)
    for f in range(NF):
        pt = psum_mm.tile([NCAT, HID], f32, space="PSUM", tag="ps")
        nc.tensor.matmul(
            out=pt[:],
            lhsT=embT_sb[:, f * NCAT:(f + 1) * NCAT],
            rhs=w1_sb[:, f, :],
            start=True, stop=True,
        )
        nc.scalar.copy(out=T_sb[:, f, :], in_=pt[:])

    # stagings for the AllGather table rows; table cols >= fdim are never
    # read by the edge matmuls, so rows are bounced as prefixes only.
    stag1 = const.tile([P, W, HID], bf16)
    stag2 = const.tile([P, W, OUT], bf16)

    NCHUNK = 4
    CW = W // NCHUNK   # 26 windows per AllGather chunk

    def bounce_chunk(stag, fdim, bounce, c):
        # stage bounce rows early, chunk by chunk, as windows finish
        lo, hi = c * CW, (c + 1) * CW
        nc.sync.dma_start(
            out=bounce.rearrange("(w p) h -> p w h", p=P)[:, lo:hi, :fdim],
            in_=stag[:, lo:hi, :])

    def ag_full(bounce, table_):
        # Shared DRAM allows a single writer: one AllGather per table
        nc.gpsimd.collective_compute(
            "AllGather", mybir.AluOpType.bypass,
            replica_groups=[list(range(NCORE))],
            ins=[bounce[:]], outs=[table_[:]],
        )

    # ---- embedding: stag1[p, w, :] = dis * sum_f onehot_f_w.T @ T_f
    for w in range(W):
        oh = oh_pool.tile([NCAT, NF, P], bf16, tag="oh")
        nc.sync.dma_start(out=oh[:], in_=onehot[w])
        pe = psum_mm.tile([P, HID], f32, space="PSUM", tag="ps")
        for f in range(NF):
            nc.tensor.matmul(
                out=pe[:], lhsT=oh[:, f, :], rhs=T_sb[:, f, :],
                start=(f == 0), stop=(f == NF - 1),
            )
        nc.scalar.activation(out=stag1[:, w, :], in_=pe[:], func=AF.Copy,
                             scale=dis_sb[:, w:w + 1])
        if (w + 1) % CW == 0:
            bounce_chunk(stag1, HID, bounce1, w // CW)
    ag_full(bounce1, table1)

    def build_s(w, engine):
        """S[p, (q,c), j] = (dstrel[p, col(q,w,c)] == j), bf16."""
        s = s_pool.tile([P, NQ * KQ, P], bf16, tag="s")
        engine.tensor_tensor(
            out=s.rearrange("p (q c) j -> p q c j", q=NQ),
            in0=iota_sb.rearrange("p (o1 o2 j) -> p o1 o2 j", o1=1, o2=1)
                .to_broadcast([P, NQ, KQ, P]),
            in1=dstrel_sb.rearrange("p (q w c) -> p q w c", q=NQ, w=W)
                [:, :, w, :]
                .rearrange("p q (c o) -> p q c o", o=1)
                .to_broadcast([P, NQ, KQ, P]),
            op=ALU.is_equal,
        )
        return s

    def edge_layer(table, fdim, epilogue, post_window=None):
        """Gather+segment-sum over all edges; call epilogue(w, psum_tile)."""
        ngrp = W // GW
        for g in range(ngrp):
            msgs = []
            for q in range(NQ):
                m = msg_pool.tile([P, GW * KQ, ROWC], bf16, tag="msg")
                c0 = (q * W + g * GW) * KQ          # first column of chunk
                nc.gpsimd.dma_gather(
                    m[:], table[BUCK * q:BUCK * (q + 1), :],
                    idx_sb[:, c0 * 8:(c0 + GW * KQ) * 8],
                    num_idxs=GW * KQ * P, num_idxs_reg=GW * KQ * P,
                    elem_size=ROWC, single_packet=False, queue_num=q,
                )
                msgs.append(m)
            for wi in range(GW):
                w = g * GW + wi
                s = build_s(w, nc.vector)
                pt = psum_mm.tile([P, fdim], f32, space="PSUM", tag="ps")
                k = 0
                for q in range(NQ):
                    for c in range(KQ):
                        nc.tensor.matmul(
                            out=pt[:],
                            lhsT=s[:, q * KQ + c, :],
                            rhs=msgs[q][:, wi * KQ + c, :fdim],
                            start=(k == 0), stop=(k == NQ * KQ - 1),
                        )
                        k += 1
                epilogue(w, pt)
                if post_window is not None:
                    post_window(w)

    def layer_norm_core(x, tag):
        """Compute LN stats of x [P, fdim] f32; return (rstd, negmurstd)."""
        stats = epi_pool.tile([P, 1, 6], f32, tag=tag + "st")
        mv = epi_pool.tile([P, 2], f32, tag=tag + "mv")
        nc.vector.bn_stats(out=stats[:, 0, :], in_=x[:])
        nc.vector.bn_aggr(out=mv[:], in_=stats[:])
        rstd = epi_pool.tile([P, 1], f32, tag=tag + "rs")
        nc.scalar.activation(out=rstd[:], in_=mv[:, 1:2], func=AF.Sqrt,
                             bias=eps_sb[:], scale=1.0)
        nc.vector.reciprocal(out=rstd[:], in_=rstd[:])
        mr = epi_pool.tile([P, 1], f32, tag=tag + "mr")
        nc.vector.tensor_tensor(out=mr[:], in0=mv[:, 0:1], in1=rstd[:],
                                op=ALU.mult)
        nmr = epi_pool.tile([P, 1], f32, tag=tag + "nm")
        nc.scalar.activation(out=nmr[:], in_=mr[:], func=AF.Copy, scale=-1.0)
        return rstd, nmr

    def epi1(w, pt):
        # out1 = dis*(psum + ht_self) + b1 ; relu
        z = epi_pool.tile([P, HID], f32, tag="z1")
        nc.vector.tensor_tensor(out=z[:], in0=pt[:], in1=stag1[:, w, :],
                                op=ALU.add)
        t = epi_pool.tile([P, HID], f32, tag="t1")
        nc.scalar.activation(out=t[:], in_=z[:], func=AF.Copy,
                             scale=dis_sb[:, w:w + 1])
        nc.vector.tensor_add(out=t[:], in0=t[:], in1=b1_sb[:])
        x = epi_pool.tile([P, HID], f32, tag="x1")
        nc.scalar.activation(out=x[:], in_=t[:], func=AF.Relu)
        # layernorm
        rstd, nmr = layer_norm_core(x, "a")
        xn = epi_pool.tile([P, HID], f32, tag="xn1")
        nc.scalar.activation(out=xn[:], in_=x[:], func=AF.Identity,
                             scale=rstd[:], bias=nmr[:])
        y = epi_pool.tile([P, HID], f32, tag="y1")
        nc.vector.tensor_tensor(out=y[:], in0=xn[:], in1=g1_sb[:],
                                op=ALU.mult)
        nc.vector.tensor_add(out=y[:], in0=y[:], in1=be1_sb[:])
        # h2 = dis * (y @ W2): transpose y then matmul (bf16)
        ptr = psum_tr.tile([HID, P], f32, space="PSUM", tag="tr")
        nc.tensor.transpose(out=ptr[:], in_=y[:], identity=ident_sb[:])
        yT = epi_pool.tile([HID, P], bf16, tag="yT")
        nc.scalar.copy(out=yT[:], in_=ptr[:])
        pw2 = psum_w2.tile([P, OUT], f32, space="PSUM", tag="w2")
        nc.tensor.matmul(out=pw2[:], lhsT=yT[:], rhs=w2_sb[:],
                         start=True, stop=True)
        nc.scalar.activation(out=stag2[:, w, :], in_=pw2[:],
                             func=AF.Copy, scale=dis_sb[:, w:w + 1])

    final = const.tile([P, W, OUT], f32)

    def epi2(w, pt):
        z = epi_pool.tile([P, OUT], f32, tag="z2")
        nc.vector.tensor_tensor(out=z[:], in0=pt[:], in1=stag2[:, w, :],
                                op=ALU.add)
        x = epi_pool.tile([P, OUT], f32, tag="x2")
        nc.scalar.activation(out=x[:], in_=z[:], func=AF.Copy,
                             scale=dis_sb[:, w:w + 1])
        nc.vector.tensor_add(out=x[:], in0=x[:], in1=b2_sb[:])
        rstd, nmr = layer_norm_core(x, "b")
        xn = epi_pool.tile([P, OUT], f32, tag="xn2")
        nc.scalar.activation(out=xn[:], in_=x[:], func=AF.Identity,
                             scale=rstd[:], bias=nmr[:])
        y = epi_pool.tile([P, OUT], f32, tag="y2")
        nc.vector.tensor_tensor(out=y[:], in0=xn[:], in1=g2_sb[:],
                                op=ALU.mult)
        nc.vector.tensor_add(out=final[:, w, :], in0=y[:], in1=be2_sb[:])

    # ---- layer 1
    def post1(w):
        # stage layer-2 bounce chunks as their windows' epilogues finish
        if (w + 1) % CW == 0:
            bounce_chunk(stag2, OUT, bounce2, w // CW)

    edge_layer(table1, HID, epi1, post_window=post1)
    ag_full(bounce2, table2)

    # ---- layer 2
    edge_layer(table2, OUT, epi2)

    nc.sync.dma_start(
        out=outx.rearrange("(w p) o -> p w o", p=P), in_=final[:])
    ctx.close()


# ------------------------------------------------------------------ host prep
def _pack_core(dloc, q_of_edge):
    """Assign local nodes to (window, slot) with per-(w,q) capacity CAP_Q and
    <=P nodes per window.  Returns win[SH], pslot[SH]."""
    cnt = np.zeros((SH, NQ), np.int64)
    np.add.at(cnt, (dloc, q_of_edge), 1)
    tot = cnt.sum(1)
    order = np.argsort(-tot, kind="stable")
    fills = np.zeros((W, NQ), np.int64)
    counts = np.zeros(W, np.int64)
    win = np.zeros(SH, np.int64)
    for n in order:
        c = cnt[n]
        ok = (counts < P) & np.all(fills + c <= CAP_Q, axis=1)
        if not ok.any():
            raise RuntimeError("window packing failed")
        load = np.where(ok[:, None], fills + c, 1 << 30).max(axis=1)
        wsel = int(np.argmin(load))
        win[n] = wsel
        fills[wsel] += c
        counts[wsel] += 1
    pslot = np.zeros(SH, np.int64)
    for wsel in range(W):
        nodes = np.nonzero(win == wsel)[0]
        pslot[nodes] = np.arange(len(nodes))
    return win, pslot


def _host_prep(x_cat, edge_index, emb_tables, W1, b1, W2, b2,
               gamma1, beta1, gamma2, beta2):
    src = np.asarray(edge_index[0], np.int64)
    dst = np.asarray(edge_index[1], np.int64)
    deg = np.bincount(dst, minlength=N).astype(np.float64) + 1.0

    core_of = np.arange(N) // SH
    wins = np.zeros(N, np.int64)
    pslots = np.zeros(N, np.int64)
    srcq = src // (2 * SH)  # bucket of an edge = pair-of-cores owning src
    for k in range(NCORE):
        m = (dst // SH) == k
        dloc = dst[m] - k * SH
        win, ps = _pack_core(dloc, srcq[m])
        wins[k * SH:(k + 1) * SH] = win
        pslots[k * SH:(k + 1) * SH] = ps
    slot_of = wins * P + pslots               # slot within owner core
    trow = core_of * SLOTS + slot_of          # global table row

    in_maps = []
    perm_slots = []
    for k in range(NCORE):
        m = (dst // SH) == k
        es, ed = src[m], dst[m] - k * SH
        ew = wins[ed + k * SH]
        ep = pslots[ed + k * SH]
        eq = trow[es] // BUCK
        # stream position: per (q, w) block of CAP_Q slots, fill in order
        gkey = eq * W + ew
        order = np.argsort(gkey, kind="stable")
        gsort = gkey[order]
        start = np.searchsorted(gsort, np.arange(NQ * W))
        rank = np.arange(len(gsort)) - start[gsort]
        assert (rank < CAP_Q).all()
        pos = gsort * CAP_Q + rank
        idx16 = np.zeros(TOTPOS, np.int16)
        drel = np.full(TOTPOS, -1.0, np.float32)
        idx16[pos] = (trow[es][order] - eq[order] * BUCK).astype(np.int16)
        drel[pos] = ep[order].astype(np.float32)
        # wrap idx: j -> [j%16, j//16], replicate x8 partition groups
        idxw = np.tile(idx16.reshape(-1, 16).T, (8, 1))
        drelw = np.ascontiguousarray(drel.reshape(-1, P).T).astype(BF16)

        # onehot [W, NCAT, NF, P] bf16 for this core's slots
        oh = np.zeros((W, NCAT, NF, P), BF16)
        sl = slot_of[k * SH:(k + 1) * SH]
        xc = np.asarray(x_cat[k * SH:(k + 1) * SH], np.int64)
        wv = sl // P
        pv = sl % P
        for f in range(NF):
            oh[wv, xc[:, f], f, pv] = 1.0

        degs = np.ones(SLOTS, np.float32)
        degs[sl] = deg[k * SH:(k + 1) * SH]
        degw = np.ascontiguousarray(degs.reshape(W, P).T)

        embT = np.ascontiguousarray(
            np.asarray(emb_tables, np.float32).transpose(2, 0, 1)
            .reshape(EMB, NF * NCAT))

        rep = lambda v, d: np.broadcast_to(
            np.asarray(v, np.float32).reshape(1, d), (P, d)).copy()

        in_maps.append({
            "onehot": oh,
            "idxs": idxw,
            "dstrel": drelw,
            "deg": degw,
            "embT": embT,
            "w1": np.ascontiguousarray(np.asarray(W1, np.float32).reshape(NF, EMB, HID).transpose(1, 0, 2)),
            "w2": np.asarray(W2, np.float32).astype(BF16),
            "b1r": rep(b1, HID), "g1r": rep(gamma1, HID),
            "be1r": rep(beta1, HID),
            "b2r": rep(b2, OUT), "g2r": rep(gamma2, OUT),
            "be2r": rep(beta2, OUT),
            "iota": np.broadcast_to(np.arange(P, dtype=np.float32), (P, P)).astype(BF16).copy(),
            "ident": np.eye(P, dtype=np.float32),
        })
        perm_slots.append(sl)
    return in_maps, perm_slots


# ------------------------------------------------------------------ entry
def kernel(x_cat, edge_index, emb_tables, W1, b1, W2, b2,
           gamma1, beta1, gamma2, beta2, _res_hook=None):
    if "nc" not in _CACHE:
        _CACHE["nc"] = build_program()
    nc = _CACHE["nc"]
    in_maps, perm_slots = _host_prep(
        np.asarray(x_cat), np.asarray(edge_index), np.asarray(emb_tables),
        np.asarray(W1), np.asarray(b1), np.asarray(W2), np.asarray(b2),
        np.asarray(gamma1), np.asarray(beta1), np.asarray(gamma2),
        np.asarray(beta2))
    res = run_bass_kernel_spmd(nc, in_maps, list(range(NCORE)),
                               **(_res_hook or {}))
    out = np.empty((N, OUT), np.float32)
    for k in range(NCORE):
        full = res.results[k]["outx"]        # [SLOTS, OUT] slot-ordered
        out[k * SH:(k + 1) * SH] = full[perm_slots[k]]
    if _res_hook is not None:
        _res_hook["result"] = res
    return out
